# revision 8
# baseline (speedup 1.0000x reference)
"""AttentiveRNNLanguageModel TRN2 kernel (8 NeuronCores, SPMD).

Strategy: embedding gather + weight-layout prep on host; on each core the
input-gate GEMM, the 512-step LSTM scan, the attention scoring and the
causal prefix-softmax pooling run replicated (the scan is inherently
sequential); the 32000-wide decoder GEMM is vocab-sharded 8 ways (4000 rows
per core, padded to 4096) and the shards are concatenated on host.

Numerics: sigma(z) = (tanh(z/2)+1)/2 so the scan needs a single ACT table
set; all the 1/2 scales are folded into the weights host-side; the cell
state is tracked as w=2c and the hidden as hh=2h. Matmuls are bf16 with
fp32 PSUM accumulation; gate tanh outputs are fp16; pooling in fp32.
"""
import sys
sys.path.insert(0, '/opt/trn_rl_repo')
from contextlib import ExitStack
import numpy as np
import ml_dtypes

import concourse.bass as bass
import concourse.bacc as bacc
import concourse.tile as tile
import concourse.mybir as mybir
from concourse.bass import ds
from concourse.bass_utils import run_bass_kernel_spmd

F32 = mybir.dt.float32
BF16 = mybir.dt.bfloat16
FP16 = mybir.dt.float16
AF = mybir.ActivationFunctionType
OP = mybir.AluOpType

E = 1024
H = 1024
G4 = 4096
B = 4
S = 512
V = 32000
NCORES = 8
VSH = 4096
T = B * S


def _host_prep(inputs):
    ids = np.asarray(inputs['input_ids']).astype(np.int64)
    emb = np.asarray(inputs['emb'], np.float32)
    W_ih = np.asarray(inputs['W_ih'], np.float32)
    W_hh = np.asarray(inputs['W_hh'], np.float32)
    b_all0 = np.asarray(inputs['b_ih'], np.float32) + np.asarray(inputs['b_hh'], np.float32)
    a1W = np.asarray(inputs['attn1_W'], np.float32)
    a1b = np.asarray(inputs['attn1_b'], np.float32)
    a2W = np.asarray(inputs['attn2_W'], np.float32)
    a2b = np.asarray(inputs['attn2_b'], np.float32)
    dW = np.asarray(inputs['dec_W'], np.float32)
    db = np.asarray(inputs['dec_b'], np.float32)
    bf = ml_dtypes.bfloat16

    xs = emb[ids]                                               # [B,S,E]
    xT = np.ascontiguousarray(xs.transpose(2, 1, 0).reshape(E, T)).astype(bf)

    gs = np.ones(G4, np.float32)
    gs[0:2 * H] = 0.5
    gs[3 * H:4 * H] = 0.5
    W_ihT = np.ascontiguousarray((W_ih * gs[:, None]).T).astype(bf)
    b_rep = np.tile((b_all0 * gs)[None, :], (128, 1)).astype(np.float32)
    W_hhT = np.ascontiguousarray((W_hh * gs[:, None] * 0.5).T).astype(bf)
    a1WT = np.ascontiguousarray((a1W * 0.5).T).astype(bf)
    b1_col = np.ascontiguousarray(a1b.reshape(8, 128).T).astype(np.float32)
    a2_col = np.ascontiguousarray(a2W.reshape(1, 8, 128)[0].T).astype(bf)
    a2b_t = a2b.reshape(1, 1).astype(np.float32)
    LT = np.triu(np.ones((S, S), np.float32)).astype(bf)
    eye4 = np.eye(4, dtype=np.float32).astype(bf)

    base = dict(xT=xT, W_ihT=W_ihT, b_rep=b_rep, W_hhT=W_hhT,
                a1WT=a1WT, b1_col=b1_col, a2_col=a2_col, a2b=a2b_t,
                LT=LT, eye4=eye4)
    in_maps = []
    for c in range(NCORES):
        lo = c * 4000
        dWc = dW[lo:lo + 4000] * 0.5
        dWc = np.concatenate([dWc, np.zeros((VSH - 4000, 2 * H), np.float32)], 0)
        dbc = np.concatenate([db[lo:lo + 4000], np.zeros(VSH - 4000, np.float32)])
        m = dict(base)
        m['dec_WT'] = np.ascontiguousarray(dWc.T).astype(bf)
        m['dec_brep'] = np.tile(dbc[None, :], (128, 1)).astype(np.float32)
        in_maps.append(m)
    return in_maps


def _build(S=S, unroll=4):
    T = B * S
    TT = T // 128          # token tiles
    SI = S // 128          # seq tiles
    CS = 4 * S + 8         # hT_all per-chunk column stride (4 zero-prefix cols)
    nc = bacc.Bacc("TRN2", target_bir_lowering=False, debug=False,
                   num_devices=NCORES)

    def din(name, shape, dt):
        return nc.dram_tensor(name, shape, dt, kind="ExternalInput").ap()

    xT_d = din("xT", [E, T], BF16)
    wih_d = din("W_ihT", [E, G4], BF16)
    brep_d = din("b_rep", [128, G4], F32)
    whh_d = din("W_hhT", [H, G4], BF16)
    a1w_d = din("a1WT", [H, H], BF16)
    b1c_d = din("b1_col", [128, 8], F32)
    a2c_d = din("a2_col", [128, 8], BF16)
    a2b_d = din("a2b", [1, 1], F32)
    lt_d = din("LT", [S, S], BF16)
    eye_d = din("eye4", [4, 4], BF16)
    dwt_d = din("dec_WT", [2 * H, VSH], BF16)
    dbr_d = din("dec_brep", [128, VSH], F32)

    logits_d = nc.dram_tensor("logits", [T, VSH], F32, kind="ExternalOutput").ap()
    xg_d = nc.dram_tensor("xg", [T, G4], BF16).ap()
    hseq_d = nc.dram_tensor("h_seq", [T, H], BF16).ap()
    e_d = nc.dram_tensor("e_buf", [T], F32).ap()

    with tile.TileContext(nc) as tc, ExitStack() as top:
        pers = top.enter_context(tc.tile_pool(name="pers", bufs=1))
        hT_all = pers.tile([128, 8, CS], BF16)
        eye4 = pers.tile([4, 4], BF16)
        nc.sync.dma_start(eye4[:], eye_d[:, :])
        for k in range(8):
            nc.vector.memset(hT_all[:, k, 0:4], 0.0)

        # ---- Stage A: xg = xT.T @ W_ihT + b ----
        with ExitStack() as ctx:
            wpool = ctx.enter_context(tc.tile_pool(name="wih", bufs=1))
            xpool = ctx.enter_context(tc.tile_pool(name="xt", bufs=1))
            bpool = ctx.enter_context(tc.tile_pool(name="bias", bufs=1))
            psA = ctx.enter_context(tc.tile_pool(name="psA", bufs=2, space="PSUM"))
            sbA = ctx.enter_context(tc.tile_pool(name="sbA", bufs=2))
            wih = wpool.tile([128, 8, G4], BF16)
            xt = xpool.tile([128, 8, T], BF16)
            bias = bpool.tile([128, G4], F32)
            for kc in range(8):
                nc.sync.dma_start(wih[:, kc, :], wih_d[128 * kc:128 * (kc + 1), :])
                nc.sync.dma_start(xt[:, kc, :], xT_d[128 * kc:128 * (kc + 1), :])
            nc.sync.dma_start(bias[:], brep_d[:, :])
            for mt in range(TT):
                for hf in range(2):
                    ps = psA.tile([128, 2048], F32)
                    for kc in range(8):
                        lhsT = xt[:, kc, 128 * mt:128 * (mt + 1)]
                        for nb in range(4):
                            nc.tensor.matmul(
                                ps[:, 512 * nb:512 * (nb + 1)], lhsT,
                                wih[:, kc, 2048 * hf + 512 * nb:2048 * hf + 512 * (nb + 1)],
                                start=(kc == 0), stop=(kc == 7))
                    sb = sbA.tile([128, 2048], BF16)
                    nc.vector.scalar_tensor_tensor(
                        sb[:], ps[:], 0.0, bias[:, 2048 * hf:2048 * (hf + 1)],
                        OP.bypass, OP.add)
                    nc.sync.dma_start(
                        xg_d[128 * mt:128 * (mt + 1), 2048 * hf:2048 * (hf + 1)], sb[:])

        # ---- Stage B: LSTM scan ----
        with ExitStack() as ctx:
            wh_pool = ctx.enter_context(tc.tile_pool(name="whh", bufs=1))
            st_pool = ctx.enter_context(tc.tile_pool(name="scanst", bufs=1))
            xg_pool = ctx.enter_context(tc.tile_pool(name="xgt", bufs=3))
            gq_ps = ctx.enter_context(tc.tile_pool(name="gq", bufs=2, space="PSUM"))
            tp_ps = ctx.enter_context(tc.tile_pool(name="tp", bufs=2, space="PSUM"))
            whh = wh_pool.tile([128, 8, G4], BF16)
            for kc in range(8):
                nc.sync.dma_start(whh[:, kc, :], whh_d[128 * kc:128 * (kc + 1), :])
            w_st = st_pool.tile([4, H], F32)
            t_sb = st_pool.tile([4, G4], FP16)
            u_t = st_pool.tile([4, H], FP16)
            v_t = st_pool.tile([4, H], F32)
            th_t = st_pool.tile([4, H], FP16)
            hh_t = st_pool.tile([4, H], BF16)
            hT_prev = st_pool.tile([128, 8, 4], BF16)
            eye4h = st_pool.tile([4, 4], FP16)
            nc.vector.memset(hT_prev[:], 0.0)
            nc.vector.memset(w_st[:], 0.0)
            nc.vector.tensor_copy(eye4h[:], eye4[:])

            def scan_body(i):
                i4 = i * 4
                xgt = xg_pool.tile([4, G4], BF16)
                nc.sync.dma_start(xgt[:], xg_d[ds(i4, 4), :])
                for q in range(4):
                    gq = gq_ps.tile([4, 1024], F32, tag="gq")
                    for nb in range(2):
                        reg = gq[:, 512 * nb:512 * (nb + 1)]
                        nc.tensor.matmul(
                            reg, eye4[:, :],
                            xgt[:, 1024 * q + 512 * nb:1024 * q + 512 * (nb + 1)],
                            start=True, stop=False)
                        for kc in range(8):
                            nc.tensor.matmul(
                                reg, hT_prev[:, kc, :],
                                whh[:, kc, 1024 * q + 512 * nb:1024 * q + 512 * (nb + 1)],
                                start=False, stop=(kc == 7))
                    nc.scalar.activation(t_sb[:, 1024 * q:1024 * (q + 1)], gq[:],
                                         AF.Tanh)
                nc.vector.scalar_tensor_tensor(u_t[:], t_sb[:, 0:H], 1.0,
                                               t_sb[:, 2 * H:3 * H], OP.add, OP.mult)
                nc.vector.scalar_tensor_tensor(v_t[:], t_sb[:, H:2 * H], 1.0,
                                               w_st[:], OP.add, OP.mult)
                nc.vector.scalar_tensor_tensor(w_st[:], v_t[:], 0.5, u_t[:],
                                               OP.mult, OP.add)
                nc.scalar.activation(th_t[:], w_st[:], AF.Tanh, scale=0.5)
                nc.vector.scalar_tensor_tensor(hh_t[:], t_sb[:, 3 * H:4 * H], 1.0,
                                               th_t[:], OP.add, OP.mult)
                nc.sync.dma_start(hseq_d[ds(i4, 4), :], hh_t[:])
                warm = gq_ps.tile([4, 1024], F32, tag="gq")
                nc.tensor.matmul(warm[:, 0:512], eye4h[:, :], u_t[:, 0:512],
                                 start=True, stop=True, skip_group_check=True)
                nc.tensor.matmul(warm[:, 512:1024], eye4h[:, :], th_t[:, 0:512],
                                 start=True, stop=True, skip_group_check=True)
                tpt = tp_ps.tile([128, 32], BF16)
                for k in range(8):
                    nc.tensor.transpose(tpt[:, 4 * k:4 * (k + 1)],
                                        hh_t[:, 128 * k:128 * (k + 1)], eye4[:, :])
                tpv = tpt.rearrange("p (k c) -> p k c", k=8)
                nc.vector.tensor_copy(hT_prev[:], tpv[:])
                nc.vector.tensor_copy(hT_all[:, :, ds(4 + i4, 4)], tpv[:])

            if unroll <= 1:
                with tc.For_i(0, S) as i:
                    scan_body(i)
            else:
                tc.For_i_unrolled(0, S, 1, scan_body, max_unroll=unroll)

        # ---- Stage C: attention + pooling ----
        ctxT = pers.tile([128, 8, S, 4], BF16)
        with ExitStack() as ctx:
            aw_pool = ctx.enter_context(tc.tile_pool(name="aw", bufs=1))
            a1_pool = ctx.enter_context(tc.tile_pool(name="a1t", bufs=1))
            psC = ctx.enter_context(tc.tile_pool(name="psC", bufs=2, space="PSUM"))
            psS = ctx.enter_context(tc.tile_pool(name="psS", bufs=1, space="PSUM"))
            sc_pool = ctx.enter_context(tc.tile_pool(name="scmisc", bufs=1))
            aw = aw_pool.tile([128, 8, H], BF16)
            for kc in range(8):
                nc.sync.dma_start(aw[:, kc, :], a1w_d[128 * kc:128 * (kc + 1), :])
            b1c = sc_pool.tile([128, 8], F32)
            nc.sync.dma_start(b1c[:], b1c_d[:, :])
            a2c = sc_pool.tile([128, 8], BF16)
            nc.sync.dma_start(a2c[:], a2c_d[:, :])
            a2b = sc_pool.tile([1, 1], F32)
            nc.sync.dma_start(a2b[:], a2b_d[:, :])

            a1t = a1_pool.tile([128, 8, T], BF16)
            for mt in range(8):
                for qf in range(T // 512):
                    ps = psC.tile([128, 512], F32)
                    for kc in range(8):
                        nc.tensor.matmul(
                            ps[:], aw[:, kc, 128 * mt:128 * (mt + 1)],
                            hT_all[:, kc, 4 + 512 * qf:4 + 512 * (qf + 1)],
                            start=(kc == 0), stop=(kc == 7))
                    nc.scalar.activation(
                        a1t[:, mt, 512 * qf:512 * (qf + 1)], ps[:], AF.Tanh,
                        bias=b1c[:, mt:mt + 1])

            e_sb = sc_pool.tile([1, T], F32)
            for qf in range(T // 512):
                ps_s = psS.tile([1, 512], F32)
                for kc in range(8):
                    nc.tensor.matmul(ps_s[:], a2c[:, kc:kc + 1],
                                     a1t[:, kc, 512 * qf:512 * (qf + 1)],
                                     start=(kc == 0), stop=(kc == 7))
                nc.scalar.activation(e_sb[:, 512 * qf:512 * (qf + 1)], ps_s[:],
                                     AF.Exp, bias=a2b[0:1, 0:1])
            nc.sync.dma_start(e_d[:], e_sb[:])
            e_col = sc_pool.tile([128, SI, 4], F32)
            nc.sync.dma_start(
                e_col[:], e_d.rearrange("(si p b) -> p si b", si=SI, p=128, b=4))
            e_cb = sc_pool.tile([128, SI, 4], BF16)
            nc.vector.tensor_copy(e_cb[:], e_col[:])

            lt_pool = ctx.enter_context(tc.tile_pool(name="lt", bufs=1))
            lt = lt_pool.tile([128, SI, S], BF16)
            for si in range(SI):
                nc.sync.dma_start(lt[:, si, :], lt_d[128 * si:128 * (si + 1), :])

            eh_pool = ctx.enter_context(tc.tile_pool(name="eh", bufs=1))
            hb_pool = ctx.enter_context(tc.tile_pool(name="hb", bufs=2))
            eh = eh_pool.tile([128, 4, SI, H], BF16)
            hs4 = hseq_d.rearrange("(s b) u -> b s u", b=4)
            for b in range(4):
                for si in range(SI):
                    hb = hb_pool.tile([128, H], BF16)
                    nc.sync.dma_start(hb[:], hs4[b, 128 * si:128 * (si + 1), :])
                    nc.vector.tensor_scalar_mul(eh[:, b, si, :], hb[:],
                                                e_col[:, si, b:b + 1])

            den_f = sc_pool.tile([1, 4, S], F32)
            psD = ctx.enter_context(tc.tile_pool(name="psD", bufs=2, space="PSUM"))
            for b in range(4):
                ps_d = psD.tile([1, S], F32, tag="d")
                for si in range(SI):
                    nc.tensor.matmul(ps_d[:, :], e_cb[:, si, b:b + 1], lt[:, si, :],
                                     start=(si == 0), stop=(si == SI - 1))
                nc.vector.tensor_copy(den_f[:, b, :], ps_d[:, :])
            rec_f = sc_pool.tile([1, 4, S], F32)
            nc.vector.reciprocal(rec_f[:], den_f[:])
            rec_fb = sc_pool.tile([1, 4, S], BF16)
            nc.vector.tensor_copy(rec_fb[:], rec_f[:])
            ones1 = sc_pool.tile([1, 128], BF16)
            nc.vector.memset(ones1[:], 1.0)
            dinv = sc_pool.tile([128, 4, S], BF16)
            for b in range(4):
                for nb in range(max(1, S // 512)):
                    w_ = min(512, S)
                    ps_b = psD.tile([128, w_], F32, tag="d")
                    nc.tensor.matmul(ps_b[:], ones1[:, :],
                                     rec_fb[:, b, w_ * nb:w_ * (nb + 1)],
                                     start=True, stop=True)
                    nc.vector.tensor_copy(dinv[:, b, w_ * nb:w_ * (nb + 1)], ps_b[:])

            psN = ctx.enter_context(tc.tile_pool(name="psN", bufs=2, space="PSUM"))
            for b in range(4):
                for uc in range(8):
                    for nb in range(max(1, S // 512)):
                        w_ = min(512, S)
                        ps_n = psN.tile([128, w_], F32)
                        for si in range(SI):
                            nc.tensor.matmul(
                                ps_n[:],
                                eh[:, b, si, 128 * uc:128 * (uc + 1)],
                                lt[:, si, w_ * nb:w_ * (nb + 1)],
                                start=(si == 0), stop=(si == SI - 1))
                        nc.vector.scalar_tensor_tensor(
                            ctxT[:, uc, w_ * nb:w_ * (nb + 1), b], ps_n[:], 0.0,
                            dinv[:, b, w_ * nb:w_ * (nb + 1)], OP.bypass, OP.mult)

        # ---- Stage D: decoder (vocab shard) ----
        with ExitStack() as ctx:
            dw_pool = ctx.enter_context(tc.tile_pool(name="dw", bufs=1))
            dbr_pool = ctx.enter_context(tc.tile_pool(name="dbr", bufs=1))
            psE = ctx.enter_context(tc.tile_pool(name="psE", bufs=2, space="PSUM"))
            ob_pool = ctx.enter_context(tc.tile_pool(name="ob", bufs=2))
            dbr = dbr_pool.tile([128, VSH], F32)
            nc.sync.dma_start(dbr[:], dbr_d[:, :])
            for vh in range(2):
                dw = dw_pool.tile([128, 16, 2048], BF16)
                for kc in range(16):
                    nc.sync.dma_start(
                        dw[:, kc, :],
                        dwt_d[128 * kc:128 * (kc + 1), 2048 * vh:2048 * (vh + 1)])
                for mt in range(TT):
                    ps = psE.tile([128, 2048], F32)
                    for kc in range(16):
                        if kc < 8:
                            lhsT = ctxT[:, kc, 32 * mt:32 * (mt + 1), :]
                        else:
                            k = kc - 8
                            lhsT = hT_all[:, k, 4 + 128 * mt:
                                          4 + 128 * (mt + 1)]
                        for nb in range(4):
                            nc.tensor.matmul(
                                ps[:, 512 * nb:512 * (nb + 1)], lhsT,
                                dw[:, kc, 512 * nb:512 * (nb + 1)],
                                start=(kc == 0), stop=(kc == 15))
                    ob = ob_pool.tile([128, 2048], F32)
                    nc.vector.scalar_tensor_tensor(
                        ob[:], ps[:], 0.0,
                        dbr[:, 2048 * vh:2048 * (vh + 1)], OP.bypass, OP.add)
                    nc.sync.dma_start(
                        logits_d[128 * mt:128 * (mt + 1),
                                 2048 * vh:2048 * (vh + 1)], ob[:])

    nc.compile()
    return nc


_NC_CACHE = {}


def _get_nc():
    if 'nc' not in _NC_CACHE:
        _NC_CACHE['nc'] = _build()
    return _NC_CACHE['nc']


def kernel(**inputs):
    nc = _get_nc()
    in_maps = _host_prep(inputs)
    res = run_bass_kernel_spmd(nc, in_maps, core_ids=list(range(NCORES)))
    shards = [res.results[c]["logits"][:, :4000] for c in range(NCORES)]
    full = np.concatenate(shards, axis=1)          # [T, 32000], rows t=4s+b
    out = full.reshape(S, B, V).transpose(1, 0, 2)  # [B, S, V]
    return np.ascontiguousarray(out)
